# revision 41
# baseline (speedup 1.0000x reference)
"""MoE-routed dynamics MLP on 8 NeuronCores.

Expert-parallel: core p holds expert p's weights. Samples are dispatched
host-side (sort by policy index), each core runs its ~B/P samples through
  concat(latent, action) [C,528] -> H=1024 (relu) -> H=1024 (relu) -> 512
with activations kept transposed ([feature, sample]) so the three GEMMs
chain on the PE without any on-chip transposes:
  h1T = relu(W1.T @ xT + b1),  h2T = relu(W2.T @ h1T + b2),
  outT = W3.T @ h2T + b3.
All matmul operands are bf16 (fp32 PSUM accumulate): same 1 PE-cycle/row
throughput as float32r but half the HBM traffic, and bf16 weights get the
compiler's Fast Weight Load so LDWEIGHTS hides under the matmuls.

DMA completions carry a multi-us pipeline/receipt lag and serialize at
~1us per transfer on the sync queue, so the supply stream is exactly 12
DMAs ordered to complete just-in-time for the K-chunk-outer / M-tile-
inner matmul schedule: [W1 chunk 0 + x pass 0] merged as ONE leading
transfer (both gate the first real matmul; merging removes a slot from
the serialized chain), W1 chunks 1-3, biases, the 16 action rows of
W1+x as one compact [16, H+C] blob (run as K=16 matmuls -- a matmul
costs N cycles regardless of K, so compact beats zero-padding to 128
rows), x pass 1, then W2 as [1H|3H|4H] and W3 as [4|4] chunk blocks
riding the post-layer-1 slack. A warmup block of dependency-free
matmuls bridges the PE from the all-engine barrier (~7us) to
first-data-ready (~11.2us) with no idle gap, so the HAM clock gate
flips to full rate at the earliest free-running window -- sizing the
bridge to the measured data-ready time is what collapses run-to-run
variance; warmup memsets are pinned to GpSimd/VectorE (nc.any would
park one behind the 1.3us ACT_TABLE_LOAD on ScalarE and stall the
first matmul).

Bias+relu ride the PSUM->SBUF eviction (ScalarE, with VectorE helping on
the final layer) writing bf16; outputs are bf16 (halves the final HBM
write + completion receipt) and flush per n-pass as [m1-m3 | m0] with
the final m0 tile issued on ScalarE's HWDGE ring in parallel with
SyncE's issue of the m1-m3 tile.
"""

import numpy as np

P = 8
D_LAT = 512
D_ACT = 16
D_IN = D_LAT + D_ACT  # 528
D_IN_PAD = 640        # 5 x 128
H = 1024
B = 4096

_compiled = {}  # capacity -> nc

# Results of the last run_bass_kernel_spmd call (for external harnesses
# that want exec_time_ns when tracing is enabled via BASS_TRACE).
LAST_RESULT = None


def _bf16(a):
    import ml_dtypes

    return np.asarray(a, dtype=np.float32).astype(ml_dtypes.bfloat16)


def _pretile(a):
    """[(k*128), F] row-major -> [128, k*F] partition-major chunks."""
    k = a.shape[0] // 128
    f = a.shape[1]
    return np.ascontiguousarray(
        a[: k * 128].reshape(k, 128, f).transpose(1, 0, 2).reshape(128, k * f)
    )


def _rep4(a):
    """[16, F] -> [128, F] with copies at partition offsets 0/32/64/96
    (zeros elsewhere), for PE row-group packed K=16 matmuls."""
    out = np.zeros((128, a.shape[1]), dtype=a.dtype)
    for g in range(4):
        out[32 * g : 32 * g + 16] = a
    return out


def _n_slices(C):
    """Split the moving (sample) dim into chunks <=512 (PSUM bank limit),
    balanced equal sizes."""
    k = -(-C // 512)
    base, rem = divmod(C, k)
    sizes = [base + (1 if i < rem else 0) for i in range(k)]
    out = []
    off = 0
    for s in sizes:
        out.append((off, s))
        off += s
    return out


def _build(C):
    import concourse.bacc as bacc
    import concourse.mybir as mybir
    import concourse.tile as tile

    f32 = mybir.dt.float32
    bf16 = mybir.dt.bfloat16
    AF = mybir.ActivationFunctionType

    nc = bacc.Bacc(None, target_bir_lowering=False)

    # Latent rows of x/W1 ship as 4 full 128-row chunks; the 16 action rows
    # of W1 and x ship together as one compact [16, H+C] blob and run as
    # K=16 matmuls -- same PE cost (matmul time is N cycles regardless of
    # K), 0.34 MB less DMA in the supply-critical layer-1 window. Each DMA
    # completion carries a ~1-3us pipeline+receipt lag, so the layer-1
    # phase uses as FEW DMAs as just-in-time streaming allows (8), and the
    # slack-rich W2/W3 ship as 4-chunk blocks.
    m1 = H // 128      # 8 M-tiles for layers 1/2
    m3 = D_LAT // 128  # 4 M-tiles for layer 3
    ns = _n_slices(C)
    nsz0 = ns[0][1]
    nj = len(ns)

    # W1 chunk 0 and pass-0 x are both needed for the very first real
    # matmul, so they ship as ONE merged DMA -- one less ~1us slot in the
    # serialized completion chain, pulling every later transfer earlier.
    wx0 = nc.declare_dram_parameter(
        "wx0", [128, H + 4 * nsz0], bf16, isOutput=False
    )
    if nj > 1:
        xn = nc.declare_dram_parameter(
            "xn", [128, 4 * nsz0 * (nj - 1)], bf16, isOutput=False
        )
    # The 16 action rows are replicated at partition offsets 0/32/64/96 so
    # the eight K=16 matmuls per pass issue to cycling PE row groups and
    # run 4-way concurrent (row-group tile packing), costing ~2 instead of
    # 8 matmul slots per pass.
    xw14 = nc.declare_dram_parameter("xw14", [128, H + C], bf16, isOutput=False)
    w15 = nc.declare_dram_parameter("w15", [128, 3 * H], bf16, isOutput=False)
    bias = nc.declare_dram_parameter("bias", [128, 20], f32, isOutput=False)
    w2 = nc.declare_dram_parameter("w2", [128, 8 * H], bf16, isOutput=False)
    w3 = nc.declare_dram_parameter("w3", [128, 8 * D_LAT], bf16, isOutput=False)
    ot = nc.declare_dram_parameter("ot", [128, 4 * C], bf16, isOutput=True)

    with tile.TileContext(nc) as tc:
        with (
            tc.tile_pool(name="xw", bufs=1) as xw,
            tc.tile_pool(name="acts", bufs=1) as acts,
            tc.tile_pool(name="psum", bufs=8, space="PSUM") as psum,
        ):
            # DMA issue order is the stream order: x, W1 chunks, bias, W2
            # chunks, W3 chunks. The Sync sequencer issues these serially
            # (~0.6us each), which keeps later transfers from competing
            # with the ones the PE needs first.
            # x arrives n-chunk-major: pass j's columns for all 5 K-chunks
            # land as one DMA, so layer 1 pass 0 starts after just w1_0+xn_0.
            # Supply order: [w1_0 + x pass-0], w1_1..3, bias, xw14,
            # xn pass-1, then W2/W3 as chunk blocks. Each piece completes
            # just in time for its K-chunk under the ~1us/DMA serialized
            # stream.
            wx0_t = xw.tile([128, H + 4 * nsz0], bf16, name="wx0_t")
            nc.sync.dma_start(out=wx0_t[:], in_=wx0[:])
            w1_t = [lambda m: wx0_t[:, m * 128 : (m + 1) * 128]]
            for k in range(1, 4):
                w = xw.tile([128, H], bf16, name=f"w1_{k}")
                nc.sync.dma_start(out=w[:], in_=w15[:, (k - 1) * H : k * H])
                w1_t.append(lambda m, _t=w: _t[:, m * 128 : (m + 1) * 128])
            bias_t = xw.tile([128, 20], f32, name="bias_t")
            nc.sync.dma_start(out=bias_t[:], in_=bias[:])
            xw14_t = xw.tile([128, H + C], bf16, name="xw14_t")
            nc.sync.dma_start(out=xw14_t[:], in_=xw14[:])
            w1_t.append(
                lambda m, _t=xw14_t: _t[
                    32 * (m % 3) : 32 * (m % 3) + 16, m * 128 : (m + 1) * 128
                ]
            )
            xn_t = []
            for j in range(1, len(ns)):
                t = xw.tile([128, 4 * nsz0], bf16, name=f"xn_{j}")
                nc.sync.dma_start(
                    out=t[:],
                    in_=xn[:, (j - 1) * 4 * nsz0 : j * 4 * nsz0],
                )
                xn_t.append(t)

            def x_at(k, n0, nsz, m):
                j = n0 // nsz
                if k == 4:
                    g = 32 * (m % 3)
                    return xw14_t[g : g + 16, H + n0 : H + n0 + nsz]
                if j == 0:
                    return wx0_t[:, H + k * nsz : H + (k + 1) * nsz]
                return xn_t[j - 1][:, k * nsz : (k + 1) * nsz]
            # W2 ships [1H | 3H | 4H]: the first K-chunk lands just in time
            # for layer 2's start; the big blocks ride the slack after it.
            w2_t = []
            for lo, hi in ((0, 1), (1, 4), (4, 8)):
                t = xw.tile([128, (hi - lo) * H], bf16, name=f"w2_{lo}")
                nc.sync.dma_start(out=t[:], in_=w2[:, lo * H : hi * H])
                for k in range(hi - lo):
                    w2_t.append(
                        lambda m, _t=t, _k=k: _t[
                            :, _k * H + m * 128 : _k * H + (m + 1) * 128
                        ]
                    )
            w3_t = []
            for half in range(2):
                t = xw.tile([128, 4 * D_LAT], bf16, name=f"w3_{half}")
                nc.sync.dma_start(
                    out=t[:], in_=w3[:, half * 4 * D_LAT : (half + 1) * 4 * D_LAT]
                )
                for k in range(4):
                    w3_t.append(
                        lambda m, _t=t, _k=k: _t[
                            :, _k * D_LAT + m * 128 : _k * D_LAT + (m + 1) * 128
                        ]
                    )

            # Warmup: bf16 matmuls with no data dependencies heat the PE
            # clock gate (HAM) while the first chunks stream in. Memsets are
            # pinned to GpSimd/Vector -- nc.any would let the scheduler put
            # one on ScalarE behind the 1.3us ACT_TABLE_LOAD, delaying the
            # first matmul by ~1.5us.
            wu_s = xw.tile([128, 128], bf16, name="wu_s")
            nc.gpsimd.memset(wu_s[:], 0.0)
            wu_m = xw.tile([128, 512], bf16, name="wu_m")
            nc.vector.memset(wu_m[:], 0.0)
            wu_p = psum.tile([128, 288], f32, tag="ps", name="wu_p")
            # 14 short + 1 long: bridges the PE from the barrier (~7us) to
            # first-data-ready (~11.2us) with no idle gap, so the HAM
            # flip fires at the earliest possible window even when the DMA
            # pipeline ramps slowly.
            for _ in range(14):
                nc.tensor.matmul(
                    wu_p[:], lhsT=wu_s[:], rhs=wu_m[:, :288], start=True, stop=True
                )
            wu_p2 = psum.tile([128, 512], f32, tag="ps", name="wu_p2")
            nc.tensor.matmul(
                wu_p2[:], lhsT=wu_s[:], rhs=wu_m[:], start=True, stop=True
            )

            # Inter-layer tiles are split per n-chunk (and the output per
            # half-pass) so consumers depend only on the slice actually
            # written -- Tile tracks deps at tile granularity, and a shared
            # [128, C] tile would make layer N+1 wait on BOTH n-passes.
            nj = len(ns)
            h1_t = [
                [acts.tile([128, nsz0], bf16, name=f"h1_{j}_{m}") for m in range(m1)]
                for j in range(nj)
            ]
            h2_t = [
                [acts.tile([128, nsz0], bf16, name=f"h2_{j}_{m}") for m in range(m1)]
                for j in range(nj)
            ]
            # Output split [m1-m3 | m0]: the last flush (m0, evicted last in
            # the reversed M order) carries only one m-tile, so the final
            # HBM write + completion receipt is as small as possible.
            o_t = [
                [
                    acts.tile([128, 3 * nsz0], bf16, name=f"o_{j}_a"),
                    acts.tile([128, nsz0], bf16, name=f"o_{j}_b"),
                ]
                for j in range(nj)
            ]

            def layer(w_tiles, rhs_at, out_at, n_m, bias_col, func, rev=False,
                      filler=0):
                """One GEMM layer, K-chunk-outer / M-tile-inner per n-pass."""
                n_k = len(w_tiles)
                morder = list(reversed(range(n_m))) if rev else list(range(n_m))
                for jn, (n0, nsz) in enumerate(ns):
                    ps = [
                        psum.tile([128, nsz], f32, tag="ps", name=f"ps{m}")
                        for m in range(n_m)
                    ]
                    for k in range(n_k):
                        for m in morder:
                            nc.tensor.matmul(
                                ps[m][:],
                                lhsT=w_tiles[k](m),
                                rhs=rhs_at(k, n0, nsz, m),
                                start=(k == 0),
                                stop=(k == n_k - 1),
                            )
                        if jn == 0 and k < 2:
                            # Zero-matmuls accumulate 0 into a live bank:
                            # numerically a no-op, but they keep the PE array
                            # busy while the next weight chunk streams in, so
                            # the HAM clock gate stays warm through layer 1's
                            # DMA-paced phase.
                            for _ in range(filler):
                                nc.tensor.matmul(
                                    ps[morder[0]][:],
                                    lhsT=wu_s[:],
                                    rhs=wu_m[:, : min(288, nsz)],
                                    start=False,
                                    stop=False,
                                )
                    for m in morder:
                        b = bias_t[:, bias_col + m : bias_col + m + 1]
                        if func == AF.Identity and m % 2 == 0:
                            nc.vector.tensor_scalar_add(
                                out_at(m, n0, nsz), ps[m][:], b
                            )
                        else:
                            nc.scalar.activation(
                                out_at(m, n0, nsz), ps[m][:], func, bias=b
                            )

            layer(
                w1_t,
                x_at,
                lambda m, n0, nsz: h1_t[n0 // nsz][m][:, :nsz],
                m1, 0, AF.Relu, filler=1,
            )
            layer(
                w2_t,
                lambda k, n0, nsz, m: h1_t[n0 // nsz][k][:, :nsz],
                lambda m, n0, nsz: h2_t[n0 // nsz][m][:, :nsz],
                m1, 8, AF.Relu,
            )
            layer(
                w3_t,
                lambda k, n0, nsz, m: h2_t[n0 // nsz][k][:, :nsz],
                lambda m, n0, nsz: o_t[n0 // nsz][1][:, :nsz]
                if m == 0
                else o_t[n0 // nsz][0][:, (m - 1) * nsz : m * nsz],
                m3, 16, AF.Identity, rev=True,
            )

            for j, (n0, nsz) in enumerate(ns):
                nc.sync.dma_start(
                    out=ot[:, 4 * n0 + nsz : 4 * n0 + 4 * nsz],
                    in_=o_t[j][0][:],
                )
                # Final pass's m0 tile goes out on ScalarE's HWDGE ring so
                # its ~0.6us issue overlaps SyncE's issue of the m1-m3 tile.
                eng = nc.scalar if j == len(ns) - 1 else nc.sync
                eng.dma_start(
                    out=ot[:, 4 * n0 : 4 * n0 + nsz],
                    in_=o_t[j][1][:],
                )

    nc.compile()
    return nc


def _ensure_axon_hooks():
    """run_bass_kernel_spmd(trace=True) imports antenv.axon_hooks, which the
    slim container lacks; provide it so tracing (e.g. BASS_TRACE=1) degrades
    gracefully or, if the ctypes hook is available, works."""
    import sys
    import types

    try:
        import antenv.axon_hooks  # noqa: F401
        return
    except ImportError:
        pass
    m = types.ModuleType("antenv.axon_hooks")
    m._hook = None
    m.set_axon_ntff_profile_hook = lambda h: setattr(m, "_hook", h)
    m.get_axon_ntff_profile_hook = lambda: m._hook
    sys.modules["antenv.axon_hooks"] = m
    try:
        from trn_agent_boot.trn_boot import _ntff_profile_via_ctypes

        m.set_axon_ntff_profile_hook(
            _ntff_profile_via_ctypes("/opt/axon/libaxon_pjrt.so")
        )
    except Exception:
        pass


def kernel(latents, actions, policy_indices, W1, b1, W2, b2, W3, b3):
    global LAST_RESULT
    _ensure_axon_hooks()
    from concourse.bass_utils import run_bass_kernel_spmd

    latents = np.ascontiguousarray(np.asarray(latents, dtype=np.float32))
    actions = np.ascontiguousarray(np.asarray(actions, dtype=np.float32))
    idx = np.asarray(policy_indices).astype(np.int64)
    W1 = np.asarray(W1, dtype=np.float32)
    b1 = np.asarray(b1, dtype=np.float32)
    W2 = np.asarray(W2, dtype=np.float32)
    b2 = np.asarray(b2, dtype=np.float32)
    W3 = np.asarray(W3, dtype=np.float32)
    b3 = np.asarray(b3, dtype=np.float32)

    n = latents.shape[0]
    order = np.argsort(idx, kind="stable")
    counts = np.bincount(idx, minlength=P)

    C = max(512, int(-(-counts.max() // 32)) * 32)
    k = -(-C // 512)
    C = -(-C // (16 * k)) * (16 * k)  # equal n-slices, width multiple of 16
    if C not in _compiled:
        _compiled[C] = _build(C)
    nc = _compiled[C]

    x = np.concatenate([latents, actions], axis=1)  # [B, 528]

    in_maps = []
    starts = np.concatenate([[0], np.cumsum(counts)])
    for p in range(P):
        sel = order[starts[p] : starts[p + 1]]
        xp = np.zeros((D_IN, C), dtype=np.float32)
        xp[:, : counts[p]] = np.ascontiguousarray(x[sel].T)
        nsl = _n_slices(C)
        xnp = np.concatenate(
            [_pretile(xp[:D_LAT, n0 : n0 + nsz]) for n0, nsz in nsl], axis=1
        )
        bp = np.concatenate(
            [
                b1[p].reshape(H // 128, 128).T,
                b2[p].reshape(H // 128, 128).T,
                b3[p].reshape(D_LAT // 128, 128).T,
            ],
            axis=1,
        )
        w15p = _pretile(W1[p][:D_LAT])
        nsz = nsl[0][1]
        im = {
            "wx0": _bf16(
                np.concatenate([w15p[:, :H], xnp[:, : 4 * nsz]], axis=1)
            ),
            "xw14": _bf16(_rep4(np.concatenate([W1[p][D_LAT:], xp[D_LAT:]], axis=1))),
            "w15": _bf16(w15p[:, H:]),
            "bias": np.ascontiguousarray(bp),
            "w2": _bf16(_pretile(W2[p])),
            "w3": _bf16(_pretile(W3[p])),
        }
        if len(nsl) > 1:
            im["xn"] = _bf16(xnp[:, 4 * nsz :])
        in_maps.append(im)

    res = run_bass_kernel_spmd(nc, in_maps, core_ids=list(range(P)))
    LAST_RESULT = res

    nsl = _n_slices(C)
    nsz = nsl[0][1]
    out = np.empty((n, D_LAT), dtype=np.float32)
    for p in range(P):
        sel = order[starts[p] : starts[p + 1]]
        # [128, n_chunks, 4, nsz] -> [D_LAT, C]
        op = (
            np.asarray(res.results[p]["ot"], dtype=np.float32)
            .reshape(128, len(nsl), 4, nsz)
            .transpose(2, 0, 1, 3)
            .reshape(D_LAT, C)
        )
        out[sel] = op[:, : counts[p]].T
    return out


# revision 42
# speedup vs baseline: 1.1957x; 1.1957x over previous
"""MoE-routed dynamics MLP on 8 NeuronCores.

Expert-parallel: core p holds expert p's weights. Samples are dispatched
host-side (sort by policy index), each core runs its ~B/P samples through
  concat(latent, action) [C,528] -> H=1024 (relu) -> H=1024 (relu) -> 512
with activations kept transposed ([feature, sample]) so the three GEMMs
chain on the PE without any on-chip transposes:
  h1T = relu(W1.T @ xT + b1),  h2T = relu(W2.T @ h1T + b2),
  outT = W3.T @ h2T + b3.
All matmul operands are bf16 (fp32 PSUM accumulate): same 1 PE-cycle/row
throughput as float32r but half the HBM traffic, and bf16 weights get the
compiler's Fast Weight Load so LDWEIGHTS hides under the matmuls.

DMA completions carry a multi-us pipeline/receipt lag and serialize at
~1us per transfer on the sync queue, so the supply stream is exactly 12
DMAs ordered to complete just-in-time for the K-chunk-outer / M-tile-
inner matmul schedule: [W1 chunk 0 + x pass 0] merged as ONE leading
transfer (both gate the first real matmul; merging removes a slot from
the serialized chain), W1 chunks 1-3, biases, the 16 action rows of
W1+x as one compact [16, H+C] blob (run as K=16 matmuls -- a matmul
costs N cycles regardless of K, so compact beats zero-padding to 128
rows), x pass 1, then W2 as [1H|3H|4H] and W3 as [4|4] chunk blocks
riding the post-layer-1 slack. A warmup block of dependency-free
matmuls bridges the PE from the all-engine barrier (~7us) to
first-data-ready (~11.2us) with no idle gap, so the HAM clock gate
flips to full rate at the earliest free-running window -- sizing the
bridge to the measured data-ready time is what collapses run-to-run
variance; warmup memsets are pinned to GpSimd/VectorE (nc.any would
park one behind the 1.3us ACT_TABLE_LOAD on ScalarE and stall the
first matmul).

Bias+relu ride the PSUM->SBUF eviction (ScalarE, with VectorE helping on
the final layer) writing bf16; outputs are bf16 (halves the final HBM
write + completion receipt) and flush per n-pass as [m1-m3 | m0] with
the final m0 tile issued on ScalarE's HWDGE ring in parallel with
SyncE's issue of the m1-m3 tile.
"""

import numpy as np

P = 8
D_LAT = 512
D_ACT = 16
D_IN = D_LAT + D_ACT  # 528
D_IN_PAD = 640        # 5 x 128
H = 1024
B = 4096

_compiled = {}  # capacity -> nc

# Results of the last run_bass_kernel_spmd call (for external harnesses
# that want exec_time_ns when tracing is enabled via BASS_TRACE).
LAST_RESULT = None


def _bf16(a):
    import ml_dtypes

    return np.asarray(a, dtype=np.float32).astype(ml_dtypes.bfloat16)


def _pretile(a):
    """[(k*128), F] row-major -> [128, k*F] partition-major chunks."""
    k = a.shape[0] // 128
    f = a.shape[1]
    return np.ascontiguousarray(
        a[: k * 128].reshape(k, 128, f).transpose(1, 0, 2).reshape(128, k * f)
    )


def _n_slices(C):
    """Split the moving (sample) dim into chunks <=512 (PSUM bank limit),
    balanced equal sizes."""
    k = -(-C // 512)
    base, rem = divmod(C, k)
    sizes = [base + (1 if i < rem else 0) for i in range(k)]
    out = []
    off = 0
    for s in sizes:
        out.append((off, s))
        off += s
    return out


def _build(C):
    import concourse.bacc as bacc
    import concourse.mybir as mybir
    import concourse.tile as tile

    f32 = mybir.dt.float32
    bf16 = mybir.dt.bfloat16
    AF = mybir.ActivationFunctionType

    nc = bacc.Bacc(None, target_bir_lowering=False)

    # Latent rows of x/W1 ship as 4 full 128-row chunks; the 16 action rows
    # of W1 and x ship together as one compact [16, H+C] blob and run as
    # K=16 matmuls -- same PE cost (matmul time is N cycles regardless of
    # K), 0.34 MB less DMA in the supply-critical layer-1 window. Each DMA
    # completion carries a ~1-3us pipeline+receipt lag, so the layer-1
    # phase uses as FEW DMAs as just-in-time streaming allows (8), and the
    # slack-rich W2/W3 ship as 4-chunk blocks.
    m1 = H // 128      # 8 M-tiles for layers 1/2
    m3 = D_LAT // 128  # 4 M-tiles for layer 3
    ns = _n_slices(C)
    nsz0 = ns[0][1]
    nj = len(ns)

    # W1 chunk 0 and pass-0 x are both needed for the very first real
    # matmul, so they ship as ONE merged DMA -- one less ~1us slot in the
    # serialized completion chain, pulling every later transfer earlier.
    wx0 = nc.declare_dram_parameter(
        "wx0", [128, H + 4 * nsz0], bf16, isOutput=False
    )
    if nj > 1:
        xn = nc.declare_dram_parameter(
            "xn", [128, 4 * nsz0 * (nj - 1)], bf16, isOutput=False
        )
    xw14 = nc.declare_dram_parameter("xw14", [16, H + C], bf16, isOutput=False)
    w15 = nc.declare_dram_parameter("w15", [128, 3 * H], bf16, isOutput=False)
    bias = nc.declare_dram_parameter("bias", [128, 20], f32, isOutput=False)
    w2 = nc.declare_dram_parameter("w2", [128, 8 * H], bf16, isOutput=False)
    w3 = nc.declare_dram_parameter("w3", [128, 8 * D_LAT], bf16, isOutput=False)
    ot = nc.declare_dram_parameter("ot", [128, 4 * C], bf16, isOutput=True)

    with tile.TileContext(nc) as tc:
        with (
            tc.tile_pool(name="xw", bufs=1) as xw,
            tc.tile_pool(name="acts", bufs=1) as acts,
            tc.tile_pool(name="psum", bufs=8, space="PSUM") as psum,
        ):
            # DMA issue order is the stream order: x, W1 chunks, bias, W2
            # chunks, W3 chunks. The Sync sequencer issues these serially
            # (~0.6us each), which keeps later transfers from competing
            # with the ones the PE needs first.
            # x arrives n-chunk-major: pass j's columns for all 5 K-chunks
            # land as one DMA, so layer 1 pass 0 starts after just w1_0+xn_0.
            # Supply order: [w1_0 + x pass-0], w1_1..3, bias, xw14,
            # xn pass-1, then W2/W3 as chunk blocks. Each piece completes
            # just in time for its K-chunk under the ~1us/DMA serialized
            # stream.
            wx0_t = xw.tile([128, H + 4 * nsz0], bf16, name="wx0_t")
            nc.sync.dma_start(out=wx0_t[:], in_=wx0[:])
            w1_t = [lambda m: wx0_t[:, m * 128 : (m + 1) * 128]]
            for k in range(1, 4):
                w = xw.tile([128, H], bf16, name=f"w1_{k}")
                nc.sync.dma_start(out=w[:], in_=w15[:, (k - 1) * H : k * H])
                w1_t.append(lambda m, _t=w: _t[:, m * 128 : (m + 1) * 128])
            bias_t = xw.tile([128, 20], f32, name="bias_t")
            nc.sync.dma_start(out=bias_t[:], in_=bias[:])
            xw14_t = xw.tile([16, H + C], bf16, name="xw14_t")
            nc.sync.dma_start(out=xw14_t[:], in_=xw14[:])
            w1_t.append(lambda m, _t=xw14_t: _t[:, m * 128 : (m + 1) * 128])
            xn_t = []
            for j in range(1, len(ns)):
                t = xw.tile([128, 4 * nsz0], bf16, name=f"xn_{j}")
                nc.sync.dma_start(
                    out=t[:],
                    in_=xn[:, (j - 1) * 4 * nsz0 : j * 4 * nsz0],
                )
                xn_t.append(t)

            def x_at(k, n0, nsz):
                j = n0 // nsz
                if k == 4:
                    return xw14_t[:, H + n0 : H + n0 + nsz]
                if j == 0:
                    return wx0_t[:, H + k * nsz : H + (k + 1) * nsz]
                return xn_t[j - 1][:, k * nsz : (k + 1) * nsz]
            # W2 ships [1H | 3H | 4H]: the first K-chunk lands just in time
            # for layer 2's start; the big blocks ride the slack after it.
            w2_t = []
            for lo, hi in ((0, 1), (1, 4), (4, 8)):
                t = xw.tile([128, (hi - lo) * H], bf16, name=f"w2_{lo}")
                nc.sync.dma_start(out=t[:], in_=w2[:, lo * H : hi * H])
                for k in range(hi - lo):
                    w2_t.append(
                        lambda m, _t=t, _k=k: _t[
                            :, _k * H + m * 128 : _k * H + (m + 1) * 128
                        ]
                    )
            w3_t = []
            for half in range(2):
                t = xw.tile([128, 4 * D_LAT], bf16, name=f"w3_{half}")
                nc.sync.dma_start(
                    out=t[:], in_=w3[:, half * 4 * D_LAT : (half + 1) * 4 * D_LAT]
                )
                for k in range(4):
                    w3_t.append(
                        lambda m, _t=t, _k=k: _t[
                            :, _k * D_LAT + m * 128 : _k * D_LAT + (m + 1) * 128
                        ]
                    )

            # Warmup: bf16 matmuls with no data dependencies heat the PE
            # clock gate (HAM) while the first chunks stream in. Memsets are
            # pinned to GpSimd/Vector -- nc.any would let the scheduler put
            # one on ScalarE behind the 1.3us ACT_TABLE_LOAD, delaying the
            # first matmul by ~1.5us.
            wu_s = xw.tile([128, 128], bf16, name="wu_s")
            nc.gpsimd.memset(wu_s[:], 0.0)
            wu_m = xw.tile([128, 512], bf16, name="wu_m")
            nc.vector.memset(wu_m[:], 0.0)
            wu_p = psum.tile([128, 288], f32, tag="ps", name="wu_p")
            # 14 short + 1 long: bridges the PE from the barrier (~7us) to
            # first-data-ready (~11.2us) with no idle gap, so the HAM
            # flip fires at the earliest possible window even when the DMA
            # pipeline ramps slowly.
            for _ in range(14):
                nc.tensor.matmul(
                    wu_p[:], lhsT=wu_s[:], rhs=wu_m[:, :288], start=True, stop=True
                )
            wu_p2 = psum.tile([128, 512], f32, tag="ps", name="wu_p2")
            nc.tensor.matmul(
                wu_p2[:], lhsT=wu_s[:], rhs=wu_m[:], start=True, stop=True
            )

            # Inter-layer tiles are split per n-chunk (and the output per
            # half-pass) so consumers depend only on the slice actually
            # written -- Tile tracks deps at tile granularity, and a shared
            # [128, C] tile would make layer N+1 wait on BOTH n-passes.
            nj = len(ns)
            h1_t = [
                [acts.tile([128, nsz0], bf16, name=f"h1_{j}_{m}") for m in range(m1)]
                for j in range(nj)
            ]
            h2_t = [
                [acts.tile([128, nsz0], bf16, name=f"h2_{j}_{m}") for m in range(m1)]
                for j in range(nj)
            ]
            # Output split [m1-m3 | m0]: the last flush (m0, evicted last in
            # the reversed M order) carries only one m-tile, so the final
            # HBM write + completion receipt is as small as possible.
            o_t = [
                [
                    acts.tile([128, 3 * nsz0], bf16, name=f"o_{j}_a"),
                    acts.tile([128, nsz0], bf16, name=f"o_{j}_b"),
                ]
                for j in range(nj)
            ]

            def layer(w_tiles, rhs_at, out_at, n_m, bias_col, func, rev=False,
                      filler=0):
                """One GEMM layer, K-chunk-outer / M-tile-inner per n-pass."""
                n_k = len(w_tiles)
                morder = list(reversed(range(n_m))) if rev else list(range(n_m))
                for jn, (n0, nsz) in enumerate(ns):
                    ps = [
                        psum.tile([128, nsz], f32, tag="ps", name=f"ps{m}")
                        for m in range(n_m)
                    ]
                    for k in range(n_k):
                        for m in morder:
                            nc.tensor.matmul(
                                ps[m][:],
                                lhsT=w_tiles[k](m),
                                rhs=rhs_at(k, n0, nsz),
                                start=(k == 0),
                                stop=(k == n_k - 1),
                            )
                        if jn == 0 and k < 2:
                            # Zero-matmuls accumulate 0 into a live bank:
                            # numerically a no-op, but they keep the PE array
                            # busy while the next weight chunk streams in, so
                            # the HAM clock gate stays warm through layer 1's
                            # DMA-paced phase.
                            for _ in range(filler):
                                nc.tensor.matmul(
                                    ps[morder[0]][:],
                                    lhsT=wu_s[:],
                                    rhs=wu_m[:, : min(288, nsz)],
                                    start=False,
                                    stop=False,
                                )
                    for m in morder:
                        b = bias_t[:, bias_col + m : bias_col + m + 1]
                        if func == AF.Identity and m % 2 == 0:
                            nc.vector.tensor_scalar_add(
                                out_at(m, n0, nsz), ps[m][:], b
                            )
                        else:
                            nc.scalar.activation(
                                out_at(m, n0, nsz), ps[m][:], func, bias=b
                            )

            layer(
                w1_t,
                x_at,
                lambda m, n0, nsz: h1_t[n0 // nsz][m][:, :nsz],
                m1, 0, AF.Relu, filler=1,
            )
            layer(
                w2_t,
                lambda k, n0, nsz: h1_t[n0 // nsz][k][:, :nsz],
                lambda m, n0, nsz: h2_t[n0 // nsz][m][:, :nsz],
                m1, 8, AF.Relu,
            )
            layer(
                w3_t,
                lambda k, n0, nsz: h2_t[n0 // nsz][k][:, :nsz],
                lambda m, n0, nsz: o_t[n0 // nsz][1][:, :nsz]
                if m == 0
                else o_t[n0 // nsz][0][:, (m - 1) * nsz : m * nsz],
                m3, 16, AF.Identity, rev=True,
            )

            for j, (n0, nsz) in enumerate(ns):
                nc.sync.dma_start(
                    out=ot[:, 4 * n0 + nsz : 4 * n0 + 4 * nsz],
                    in_=o_t[j][0][:],
                )
                # Final pass's m0 tile goes out on ScalarE's HWDGE ring so
                # its ~0.6us issue overlaps SyncE's issue of the m1-m3 tile.
                eng = nc.scalar if j == len(ns) - 1 else nc.sync
                eng.dma_start(
                    out=ot[:, 4 * n0 : 4 * n0 + nsz],
                    in_=o_t[j][1][:],
                )

    nc.compile()
    return nc


def _ensure_axon_hooks():
    """run_bass_kernel_spmd(trace=True) imports antenv.axon_hooks, which the
    slim container lacks; provide it so tracing (e.g. BASS_TRACE=1) degrades
    gracefully or, if the ctypes hook is available, works."""
    import sys
    import types

    try:
        import antenv.axon_hooks  # noqa: F401
        return
    except ImportError:
        pass
    m = types.ModuleType("antenv.axon_hooks")
    m._hook = None
    m.set_axon_ntff_profile_hook = lambda h: setattr(m, "_hook", h)
    m.get_axon_ntff_profile_hook = lambda: m._hook
    sys.modules["antenv.axon_hooks"] = m
    try:
        from trn_agent_boot.trn_boot import _ntff_profile_via_ctypes

        m.set_axon_ntff_profile_hook(
            _ntff_profile_via_ctypes("/opt/axon/libaxon_pjrt.so")
        )
    except Exception:
        pass


def kernel(latents, actions, policy_indices, W1, b1, W2, b2, W3, b3):
    global LAST_RESULT
    _ensure_axon_hooks()
    from concourse.bass_utils import run_bass_kernel_spmd

    latents = np.ascontiguousarray(np.asarray(latents, dtype=np.float32))
    actions = np.ascontiguousarray(np.asarray(actions, dtype=np.float32))
    idx = np.asarray(policy_indices).astype(np.int64)
    W1 = np.asarray(W1, dtype=np.float32)
    b1 = np.asarray(b1, dtype=np.float32)
    W2 = np.asarray(W2, dtype=np.float32)
    b2 = np.asarray(b2, dtype=np.float32)
    W3 = np.asarray(W3, dtype=np.float32)
    b3 = np.asarray(b3, dtype=np.float32)

    n = latents.shape[0]
    order = np.argsort(idx, kind="stable")
    counts = np.bincount(idx, minlength=P)

    C = max(512, int(-(-counts.max() // 32)) * 32)
    k = -(-C // 512)
    C = -(-C // (16 * k)) * (16 * k)  # equal n-slices, width multiple of 16
    if C not in _compiled:
        _compiled[C] = _build(C)
    nc = _compiled[C]

    x = np.concatenate([latents, actions], axis=1)  # [B, 528]

    in_maps = []
    starts = np.concatenate([[0], np.cumsum(counts)])
    for p in range(P):
        sel = order[starts[p] : starts[p + 1]]
        xp = np.zeros((D_IN, C), dtype=np.float32)
        xp[:, : counts[p]] = np.ascontiguousarray(x[sel].T)
        nsl = _n_slices(C)
        xnp = np.concatenate(
            [_pretile(xp[:D_LAT, n0 : n0 + nsz]) for n0, nsz in nsl], axis=1
        )
        bp = np.concatenate(
            [
                b1[p].reshape(H // 128, 128).T,
                b2[p].reshape(H // 128, 128).T,
                b3[p].reshape(D_LAT // 128, 128).T,
            ],
            axis=1,
        )
        w15p = _pretile(W1[p][:D_LAT])
        nsz = nsl[0][1]
        im = {
            "wx0": _bf16(
                np.concatenate([w15p[:, :H], xnp[:, : 4 * nsz]], axis=1)
            ),
            "xw14": _bf16(
                np.concatenate([W1[p][D_LAT:], xp[D_LAT:]], axis=1)
            ),
            "w15": _bf16(w15p[:, H:]),
            "bias": np.ascontiguousarray(bp),
            "w2": _bf16(_pretile(W2[p])),
            "w3": _bf16(_pretile(W3[p])),
        }
        if len(nsl) > 1:
            im["xn"] = _bf16(xnp[:, 4 * nsz :])
        in_maps.append(im)

    res = run_bass_kernel_spmd(nc, in_maps, core_ids=list(range(P)))
    LAST_RESULT = res

    nsl = _n_slices(C)
    nsz = nsl[0][1]
    out = np.empty((n, D_LAT), dtype=np.float32)
    for p in range(P):
        sel = order[starts[p] : starts[p + 1]]
        # [128, n_chunks, 4, nsz] -> [D_LAT, C]
        op = (
            np.asarray(res.results[p]["ot"], dtype=np.float32)
            .reshape(128, len(nsl), 4, nsz)
            .transpose(2, 0, 1, 3)
            .reshape(D_LAT, C)
        )
        out[sel] = op[:, : counts[p]].T
    return out
